# revision 21
# baseline (speedup 1.0000x reference)
"""Trainium2 8-core SPMD kernel for PointConvNet2 (gnn_message_passing).

Strategy (dst-grid, node-sharded):
- Self loops added; each edge owned by the core owning its dst node.
- Nodes assigned to cores round-robin by global degree-desc rank, so every
  core has an (equalized) identical round profile -> ONE SPMD program.
- Per core, edges laid out in "rounds": round r = the r-th edge of every
  node with deg > r, ordered by local rank (= slab column). Rounds padded
  to multiples of 512 with duplicate edges (max() no-ops).
- Device per 512-edge tile: indirect-gather 4x[128,40] fp16 rows of the
  packed x|pos table, subtract resident dst positions (static SBUF slices),
  PE-transpose to [40,512], evac, mm1 (K=35), fused relu+b1 evac, mm2
  (K=64), DVE max into a resident [64, C_SLAB] fp32 slab at static column
  offsets. No dst indices ever reach the device.
- Host gathers the 8 slabs and unpermutes ranks -> node ids.

Wait-budget discipline: this walrus build allows at most ONE sync wait per
compute-engine instruction. All constants ship in ONE dma (one semaphore),
each engine runs a one-time "priming" op observing that sem, and a tiny DVE
absorber before each slab-max keeps every instruction at <=1 wait.
"""

import numpy as np

N = 100000
E_EDGES = 1600000
NC = 8
C_NODES = N // NC  # 12500
TILE = 512
D_FEAT = 32
D_IN = 35
D_HID = 64
D_OUT = 64
XP_COLS = 40  # x(32) | pos(3) | pad(5) = 80B fp16 rows

# const blob layout (fp16 columns)
_IDENT0 = 0  # [128, 128] identity f16
_W10 = 128  # [35, 64]
_W20 = 192  # [64, 64]
_B10 = 256  # [64, 2] = f32 [64, 1]
_B20 = 258  # [64, 2]
_POS0 = 260  # [128, 3*nblk]


# ---------------------------------------------------------------- host prep
def _preprocess(edge_index):
    # self loops FIRST: stable sort then makes each node's round-0 edge its
    # self loop, letting round 0 ship resident (no indirect gathers)
    src = np.concatenate([np.arange(N), edge_index[0]]).astype(np.int64)
    dst = np.concatenate([np.arange(N), edge_index[1]]).astype(np.int64)

    deg = np.bincount(dst, minlength=N)  # >=1 via self loops
    order = np.argsort(-deg, kind="stable")  # order[g] = node at global rank g
    rank_of = np.empty(N, dtype=np.int64)
    rank_of[order] = np.arange(N)

    g = rank_of[dst]
    core = (g % NC).astype(np.int64)
    k = g // NC  # local rank = slab column

    deg_rank = deg[order]
    deg_by_core = deg_rank.reshape(C_NODES, NC).T  # [NC, C_NODES]
    R = int(deg.max())
    nr = np.zeros((NC, R), dtype=np.int64)
    for c in range(NC):
        cnt = np.bincount(deg_by_core[c], minlength=R + 1)
        suf = np.cumsum(cnt[::-1])[::-1]
        nr[c] = suf[1 : R + 1]
    nr_common = nr.max(axis=0)
    nr_pad = ((nr_common + TILE - 1) // TILE) * TILE
    round_start = np.concatenate([[0], np.cumsum(nr_pad)])
    E_pad = int(round_start[-1])
    T_tiles = E_pad // TILE
    C_SLAB = int(nr_pad.max())

    tile_col0 = np.concatenate(
        [np.arange(0, nr_pad[r], TILE) for r in range(R)]
    ).astype(np.int64)
    assert len(tile_col0) == T_tiles

    e_order = np.argsort(dst, kind="stable")
    dst_sorted = dst[e_order]
    node_starts = np.concatenate([[0], np.cumsum(deg)])
    j_sorted = np.arange(len(dst_sorted)) - node_starts[dst_sorted]

    src_grid = np.full((NC, E_pad), -1, dtype=np.int64)
    pos_sorted = round_start[j_sorted] + k[e_order]
    src_grid[core[e_order], pos_sorted] = src[e_order]

    cols = np.concatenate([np.arange(nr_pad[r]) for r in range(R)])
    for c in range(NC):
        gc = src_grid[c]
        empty = gc < 0
        fill = np.where(cols < C_NODES, gc[np.minimum(cols, C_NODES - 1)], 0)
        gc[empty] = fill[empty]

    # per-partition-major idx stream: [128, T, 4] int32 per core
    idx = (
        src_grid.reshape(NC, T_tiles, 4, 128)
        .transpose(0, 3, 1, 2)
        .astype(np.int32)
        .copy()
    )

    node_by_core_rank = order.reshape(C_NODES, NC).T  # [NC, C_NODES]

    return dict(
        idx=idx,
        tile_col0=tile_col0,
        T_tiles=T_tiles,
        C_SLAB=C_SLAB,
        node_by_core_rank=node_by_core_rank,
        src_r0=src_grid[:, : C_SLAB].copy(),  # round-0 srcs (= self loops)
    )


def _build_tables(x, pos, meta):
    xp = np.zeros((N, XP_COLS), dtype=np.float16)
    xp[:, :D_FEAT] = x.astype(np.float16)
    xp[:, D_FEAT : D_FEAT + 3] = pos.astype(np.float16)

    C_SLAB = meta["C_SLAB"]
    posT = np.zeros((NC, 3, C_SLAB), dtype=np.float16)
    for c in range(NC):
        nodes = meta["node_by_core_rank"][c]
        posT[c, :, : len(nodes)] = pos[nodes].astype(np.float16).T
    return xp, posT


def _blob_geometry(T_tiles, C_SLAB):
    idx0 = _POS0
    pt0 = idx0 + T_tiles * 8  # T*4 int32 = T*8 f16 cols
    xr0 = pt0 + C_SLAB  # posT-pad section [35 rows used, C_SLAB]
    t_r0 = C_SLAB // TILE  # round-0 tiles, resident xp rows
    blob_cols = xr0 + t_r0 * 4 * XP_COLS
    return idx0, pt0, xr0, blob_cols


def _pack_blob(meta, posT_c, idx_c, W1, b1, W2, b2, xr_c):
    T = meta["T_tiles"]
    idx0, pt0, xr0, blob_cols = _blob_geometry(T, meta["C_SLAB"])
    blob = np.zeros((128, blob_cols), dtype=np.uint16)
    blob[:, _IDENT0 : _IDENT0 + 128] = np.eye(128, dtype=np.float16).view(np.uint16)
    blob[:D_IN, _W10 : _W10 + D_HID] = W1.astype(np.float16).view(np.uint16)
    blob[:D_HID, _W20 : _W20 + D_OUT] = W2.astype(np.float16).view(np.uint16)
    blob[:D_HID, _B10 : _B10 + 2] = (
        b1.astype(np.float32).reshape(D_HID, 1).view(np.uint16)
    )
    blob[:D_OUT, _B20 : _B20 + 2] = (
        b2.astype(np.float32).reshape(D_OUT, 1).view(np.uint16)
    )
    blob[:, idx0 : idx0 + T * 8] = (
        idx_c.reshape(128, T * 4).view(np.uint16)
    )
    # posT-pad: rows 32:35 carry dst positions by rank; other rows zero
    blob[D_FEAT : D_FEAT + 3, pt0 : pt0 + meta["C_SLAB"]] = posT_c.view(np.uint16)
    # round-0 resident xp rows: [p, t*160 + j*40 + d]
    blob[:, xr0 : xr0 + xr_c.shape[1]] = xr_c
    return blob


# ------------------------------------------------------------- bass program
def _build_program(T_tiles, tile_col0, C_SLAB):
    """Raw-bass SPMD program (Tile framework output does not compile on this
    walrus build: >1 sync wait per instruction is rejected, and Tile emits
    multi-wait instructions everywhere incl. its tail drain). Explicit
    per-engine streams with a software pipeline; every instruction carries
    at most one inline wait, standalone EVSEM waits hold up to two."""
    from contextlib import ExitStack

    import concourse.bass as bass
    import concourse.mybir as mybir

    f16 = mybir.dt.float16
    f32 = mybir.dt.float32
    i32 = mybir.dt.int32
    u16 = mybir.dt.uint16
    idx0, pt0, xr0, blob_cols = _blob_geometry(T_tiles, C_SLAB)
    T_R0 = C_SLAB // TILE
    NB = 2  # psum/msgT/h1 ring depth
    NB_G = 4  # gather ring depth

    nc = bass.Bass()
    xp_d = nc.declare_dram_parameter("xp", [N, XP_COLS], f16, isOutput=False)
    blob_d = nc.declare_dram_parameter("blob", [128, blob_cols], u16, isOutput=False)
    out_d = nc.declare_dram_parameter("out", [D_OUT, C_SLAB], f32, isOutput=True)

    Relu = mybir.ActivationFunctionType.Relu
    SUB = mybir.AluOpType.subtract
    MAX = mybir.AluOpType.max
    ADD = mybir.AluOpType.add

    # slab_dep[t]: required s_max value before slabmax(t) may run: the
    # same-column writer in the previous round (+1 for its inc, +1 for the
    # memset inc), or just the memset (1) for round-0 tiles.
    slab_dep = []
    round_first = 0  # tile index where current round starts
    prev_round_first = None
    for t in range(T_tiles):
        if t > 0 and int(tile_col0[t]) == 0:
            prev_round_first = round_first
            round_first = t
        if prev_round_first is None or round_first == 0:
            slab_dep.append(1)
        else:
            pw = prev_round_first + int(tile_col0[t]) // TILE
            slab_dep.append(pw + 2)

    with ExitStack() as ctx:
        e = ctx.enter_context
        blob = e(nc.sbuf_tensor("blob_sb", [128, blob_cols], u16))
        slab = e(nc.sbuf_tensor("slab", [D_OUT, C_SLAB], f32))
        gath = [e(nc.sbuf_tensor(f"gath{i}", [128, 4, XP_COLS], f16)) for i in range(NB_G)]
        msgT = [e(nc.sbuf_tensor(f"msgT{i}", [D_IN, TILE], f16)) for i in range(NB)]
        h1 = [e(nc.sbuf_tensor(f"h1_{i}", [D_HID, TILE], f16)) for i in range(NB)]
        psumT = [e(nc.psum_tensor(f"psumT{i}", [XP_COLS, TILE], f16)) for i in range(NB)]
        psum1 = [e(nc.psum_tensor(f"psum1_{i}", [D_HID, TILE], f32)) for i in range(NB)]
        psum2 = [e(nc.psum_tensor(f"psum2_{i}", [D_OUT, TILE], f32)) for i in range(NB)]

        s_blob = e(nc.semaphore("s_blob"))
        s_g = [e(nc.semaphore(f"s_g{i}")) for i in range(NB_G)]
        s_tr = e(nc.semaphore("s_tr"))
        s_mm1 = e(nc.semaphore("s_mm1"))
        s_mm2 = e(nc.semaphore("s_mm2"))
        s_ev = e(nc.semaphore("s_ev"))
        s_relu = e(nc.semaphore("s_relu"))
        s_max = e(nc.semaphore("s_max"))
        s_fin = e(nc.semaphore("s_fin"))

        ident = blob[:, _IDENT0 : _IDENT0 + 128].bitcast(f16)
        w1 = blob[:D_IN, _W10 : _W10 + D_HID].bitcast(f16)
        w2 = blob[:D_HID, _W20 : _W20 + D_OUT].bitcast(f16)
        b1 = blob[:D_HID, _B10 : _B10 + 2].bitcast(f32)
        b2 = blob[:D_OUT, _B20 : _B20 + 2].bitcast(f32)
        idxv = blob[:, idx0 : idx0 + T_tiles * 8].bitcast(i32)  # [128, T*4]
        posTpad = blob[:D_IN, pt0 : pt0 + C_SLAB].bitcast(f16)  # rows 32:35
        xrv = blob[:, xr0 : xr0 + T_R0 * 4 * XP_COLS].bitcast(f16)

        with nc.Block() as block:

            @block.sync
            def _(sp):
                sp.dma_start(out=blob[:, :], in_=blob_d[:, :]).then_inc(s_blob, 16)
                sp.dma_start(out=out_d[:, :], in_=slab[:, :])._wait_ge(s_fin, 1).then_inc(s_fin, 16)

            @block.gpsimd
            def _(gp):
                gp.wait_ge(s_blob, 16)
                for t in range(T_R0, T_tiles):
                    for j in range(4):
                        i = gp.indirect_dma_start(
                            out=gath[t % NB_G][:, j, :],
                            out_offset=None,
                            in_=xp_d[:, :],
                            in_offset=bass.IndirectOffsetOnAxis(
                                ap=idxv[:, t * 4 + j : t * 4 + j + 1], axis=0
                            ),
                        )
                        if j == 0 and t >= T_R0 + NB_G:
                            # WAR: transposes of tile t-NB_G read the slot; the
                            # Pool sequencer blocks here, gating j=1..3 too
                            i._wait_ge(s_tr, t - NB_G + 1)
                        i.then_inc(s_g[t % NB_G], 16)

            @block.tensor
            def _(pe):
                pe.wait_ge(s_blob, 16)
                n_gath = [0] * NB_G  # emitted gathers per slot
                for t in range(T_tiles):
                    # ---- transposes of tile t
                    if t < T_R0:
                        # round 0 resident in blob; only psumT WAR needed
                        if t >= NB:
                            pe.wait_ge(s_ev, t - NB + 1)
                    else:
                        n_gath[t % NB_G] += 1
                        w = pe.wait_ge(s_g[t % NB_G], 64 * n_gath[t % NB_G])
                        if t >= NB:  # psumT WAR vs evac(t-NB)
                            w.wait_op(s_ev, t - NB + 1, "sem-ge")
                    last = None
                    for j in range(4):
                        src_ap = (
                            xrv[:, (t * 4 + j) * XP_COLS : (t * 4 + j + 1) * XP_COLS]
                            if t < T_R0
                            else gath[t % NB_G][:, j, :]
                        )
                        last = nc.tensor.transpose(
                            out=psumT[t % NB][:, j * 128 : (j + 1) * 128],
                            in_=src_ap,
                            identity=ident,
                        )
                    last.then_inc(s_tr, 1)
                    # ---- mm1(t-1), mm2(t-1) (software pipelined)
                    if t >= 1:
                        u = t - 1
                        w = pe.wait_ge(s_ev, u + 1)
                        if u >= NB:  # psum1 WAR vs relu(u-NB)
                            w.wait_op(s_relu, u - NB + 1, "sem-ge")
                        nc.tensor.matmul(
                            out=psum1[u % NB][:, :], lhsT=w1,
                            rhs=msgT[u % NB][:, :], start=True, stop=True,
                        ).then_inc(s_mm1, 1)
                        w = pe.wait_ge(s_relu, u + 1)
                        if u >= NB:  # psum2 WAR vs slabmax(u-NB)
                            w.wait_op(s_max, u - NB + 1, "sem-ge")
                        nc.tensor.matmul(
                            out=psum2[u % NB][:, :], lhsT=w2,
                            rhs=h1[u % NB][:, :], start=True, stop=True,
                        ).then_inc(s_mm2, 1)
                # tail: mm1/mm2 of last tile
                u = T_tiles - 1
                w = pe.wait_ge(s_ev, u + 1)
                w.wait_op(s_relu, u - NB + 1, "sem-ge")
                nc.tensor.matmul(
                    out=psum1[u % NB][:, :], lhsT=w1,
                    rhs=msgT[u % NB][:, :], start=True, stop=True,
                ).then_inc(s_mm1, 1)
                w = pe.wait_ge(s_relu, u + 1)
                w.wait_op(s_max, u - NB + 1, "sem-ge")
                nc.tensor.matmul(
                    out=psum2[u % NB][:, :], lhsT=w2,
                    rhs=h1[u % NB][:, :], start=True, stop=True,
                ).then_inc(s_mm2, 1)

            @block.scalar
            def _(act):
                act.wait_ge(s_blob, 16)
                for t in range(T_tiles):
                    w = act.wait_ge(s_mm1, t + 1)
                    if t >= NB:  # h1 WAR vs mm2(t-NB)
                        w.wait_op(s_mm2, t - NB + 1, "sem-ge")
                    nc.scalar.activation(
                        out=h1[t % NB][:, :], in_=psum1[t % NB][:, :],
                        func=Relu, bias=b1,
                    ).then_inc(s_relu, 1)

            @block.vector
            def _(dv):
                nc.vector.memset(slab[:, :], -1e30).then_inc(s_max, 1)
                dv.wait_ge(s_blob, 16)
                for t in range(T_tiles):
                    # evac+dp of tile t
                    col0 = int(tile_col0[t])
                    w = dv.wait_ge(s_tr, t + 1)
                    if t >= NB:  # msgT WAR vs mm1(t-NB)
                        w.wait_op(s_mm1, t - NB + 1, "sem-ge")
                    nc.vector.tensor_tensor(
                        out=msgT[t % NB][:, :],
                        in0=psumT[t % NB][:D_IN, :],
                        in1=posTpad[:, col0 : col0 + TILE],
                        op=SUB,
                    ).then_inc(s_ev, 1)
                    # slabmax of tile t-1
                    if t >= 1:
                        u = t - 1
                        w = dv.wait_ge(s_mm2, u + 1)
                        # same-col writer in the previous round (or memset)
                        w.wait_op(s_max, slab_dep[u], "sem-ge")
                        nc.vector.tensor_tensor(
                            out=slab[:, int(tile_col0[u]) : int(tile_col0[u]) + TILE],
                            in0=slab[:, int(tile_col0[u]) : int(tile_col0[u]) + TILE],
                            in1=psum2[u % NB][:, :],
                            op=MAX,
                        ).then_inc(s_max, 1)
                u = T_tiles - 1
                w = dv.wait_ge(s_mm2, u + 1)
                w.wait_op(s_max, slab_dep[u], "sem-ge")
                nc.vector.tensor_tensor(
                    out=slab[:, int(tile_col0[u]) : int(tile_col0[u]) + TILE],
                    in0=slab[:, int(tile_col0[u]) : int(tile_col0[u]) + TILE],
                    in1=psum2[u % NB][:, :],
                    op=MAX,
                ).then_inc(s_max, 1)
                # final bias add, then release the store
                nc.vector.tensor_scalar(
                    out=slab[:, :], in0=slab[:, :], scalar1=b2, scalar2=None,
                    op0=ADD,
                )._wait_ge(s_max, T_tiles + 1).then_inc(s_fin, 1)

    return nc


def check_waits(nc, max_show=8):
    """Report engine instructions with >1 sync wait (walrus limit)."""
    import json
    import bass_rust

    m = json.loads(bass_rust.module_to_json_string(nc.m))
    bad = []
    for f in m["functions"]:
        for blk in f["blocks"]:
            for inst in blk["instructions"]:
                op = inst["opcode"]
                if op in ("Drain", "EventSemaphore", "Call", "UnconditionalBranch",
                          "RegisterMove", "ISA", "DMACopy"):
                    continue
                w = (inst.get("sync_info") or {}).get("on_wait") or []
                if len(w) > 1:
                    bad.append((op, inst.get("name"),
                                [(x["ant_name"], x["wait_value"]) for x in w]))
    for b in bad[:max_show]:
        print("  MULTIWAIT:", b)
    return bad


# ------------------------------------------------------------------ runner
_CACHE = {}


def _get_program(meta):
    key = (meta["T_tiles"], meta["C_SLAB"], tuple(meta["tile_col0"][:8]))
    if key not in _CACHE:
        _CACHE[key] = _build_program(
            meta["T_tiles"], meta["tile_col0"], meta["C_SLAB"]
        )
    return _CACHE[key]


def _make_in_maps(inputs, meta):
    xp, posT = _build_tables(
        np.asarray(inputs["x"]), np.asarray(inputs["pos"]), meta
    )
    maps = []
    C_SLAB = meta["C_SLAB"]
    t_r0 = C_SLAB // TILE
    for c in range(NC):
        xr = (
            xp[meta["src_r0"][c]]  # [C_SLAB, 40] fp16
            .reshape(t_r0, 4, 128, XP_COLS)
            .transpose(2, 0, 1, 3)
            .reshape(128, t_r0 * 4 * XP_COLS)
            .view(np.uint16)
        )
        maps.append(
            dict(
                xp=xp,
                blob=_pack_blob(
                    meta, posT[c], meta["idx"][c],
                    np.asarray(inputs["W1"]), np.asarray(inputs["b1"]),
                    np.asarray(inputs["W2"]), np.asarray(inputs["b2"]), xr,
                ),
            )
        )
    return maps


def _assemble(results, meta):
    out_full = np.empty((N, D_OUT), dtype=np.float32)
    for c in range(NC):
        slab = results[c]["out"]  # [64, C_SLAB]
        nodes = meta["node_by_core_rank"][c]
        out_full[nodes] = slab[:, :C_NODES].T
    return out_full


def kernel(x, pos, edge_index, W1, b1, W2, b2):
    from concourse.bass_utils import run_bass_kernel_spmd

    meta = _preprocess(np.asarray(edge_index))
    nc = _get_program(meta)
    in_maps = _make_in_maps(
        dict(x=x, pos=pos, W1=W1, b1=b1, W2=W2, b2=b2), meta
    )
    res = run_bass_kernel_spmd(nc, in_maps, core_ids=list(range(NC)))
    return _assemble(res.results, meta)


def _sharded_fn(nc, in_maps):
    """Build the sharded jit + device-resident inputs for steady-state
    timing (mirrors bass2jax.run_bass_via_pjrt without donation)."""
    import jax
    import jax.numpy as jnp
    from jax.sharding import Mesh, PartitionSpec
    from jax.experimental.shard_map import shard_map
    import concourse.mybir as mybir
    from concourse import bass2jax

    bass2jax.install_neuronx_cc_hook()
    m = nc.m
    in_names, out_names, out_avals = [], [], []
    partition_name = nc.partition_id_tensor.name if nc.partition_id_tensor else None
    for alloc in m.functions[0].allocations:
        if not isinstance(alloc, mybir.MemoryLocationSet):
            continue
        name = alloc.memorylocations[0].name
        if alloc.kind == "ExternalInput":
            if name != partition_name:
                in_names.append(name)
        elif alloc.kind == "ExternalOutput":
            out_names.append(name)
            out_avals.append(
                jax.core.ShapedArray(
                    tuple(alloc.tensor_shape), mybir.dt.np(alloc.dtype)
                )
            )
    n_params = len(in_names)
    zero_outs = [np.zeros(a.shape, a.dtype) for a in out_avals]
    all_in = in_names + out_names
    if partition_name is not None:
        all_in.append(partition_name)

    def _body(*args):
        operands = list(args)
        if partition_name is not None:
            operands.append(bass2jax.partition_id_tensor())
        return tuple(
            bass2jax._bass_exec_p.bind(
                *operands,
                out_avals=tuple(out_avals),
                in_names=tuple(all_in),
                out_names=tuple(out_names),
                lowering_input_output_aliases=(),
                sim_require_finite=True,
                sim_require_nnan=True,
                nc=nc,
            )
        )

    devices = jax.devices()[:NC]
    mesh = Mesh(np.asarray(devices), ("core",))
    n_outs = len(out_avals)
    sharded = jax.jit(
        shard_map(
            _body, mesh=mesh,
            in_specs=(PartitionSpec("core"),) * (n_params + n_outs),
            out_specs=(PartitionSpec("core"),) * n_outs,
            check_rep=False,
        ),
        keep_unused=True,
    )
    sh = jax.sharding.NamedSharding(mesh, PartitionSpec("core"))
    dev_in = [
        jax.device_put(
            np.concatenate([np.asarray(im[n]) for im in in_maps], axis=0), sh
        )
        for n in in_names
    ] + [
        jax.device_put(
            np.zeros((NC * z.shape[0], *z.shape[1:]), z.dtype), sh
        )
        for z in zero_outs
    ]
    return sharded, dev_in


def bench(inputs, iters=10):
    """Steady-state wall-clock of the sharded executable (ns)."""
    import time
    import jax

    meta = _preprocess(np.asarray(inputs["edge_index"]))
    nc = _get_program(meta)
    in_maps = _make_in_maps(inputs, meta)
    fn, dev_in = _sharded_fn(nc, in_maps)
    jax.block_until_ready(fn(*dev_in))  # compile+warm
    times = []
    for _ in range(iters):
        t0 = time.perf_counter()
        jax.block_until_ready(fn(*dev_in))
        times.append(time.perf_counter() - t0)
    ts = sorted(times)
    print("bench walls (us):", [round(t * 1e6) for t in ts])
    return int(ts[0] * 1e9)


if __name__ == "__main__":
    d = np.load("/root/problem/inputs_cache.npz")
    out = kernel(
        d["x"], d["pos"], d["edge_index"], d["W1"], d["b1"], d["W2"], d["b2"]
    )
    np.save("/root/problem/kernel_out.npy", out)
    print("kernel output", out.shape, out.dtype)
